# Initial kernel scaffold
#
"""NTM-style scatter-memory kernel for Trainium2, 8 NeuronCores.

Strategy (K-sharded matmul + batch-sharded memory update):
  - The three Dense matmuls x @ [Wk|We|Wa] (K = 12416) are sharded over the
    contraction dim: core c holds rows [1552c, 1552(c+1)) of the weights and
    the matching columns of x^T, computes a partial [256, 1152] product, and a
    ReduceScatter(+) hands core c the full-sum rows for its 32 batches.
    This avoids replicating the ~19 MB weight matrices on every core.
  - Everything else (cosine addressing, softmax, erase/add, memory update) is
    batch-local: core c owns batches [32c, 32c+32).
Host-side: shard/concat/cast prep in numpy; device kernel does all FLOPs.
"""

import numpy as np
import ml_dtypes

import bass_rust
import concourse.bass as bass
import concourse.tile as tile
from concourse import mybir
from concourse.bass_utils import run_bass_kernel_spmd
from concourse.vector_clock import ScopedClock

# ---------------------------------------------------------------- dimensions
B, H, N, M, D_IN = 256, 6, 2048, 64, 128
D_CAT = H * N + D_IN      # 12416
D_OUT = H * M             # 384
NCORES = 8
KS = D_CAT // NCORES      # 1552 contraction rows per core
BS = B // NCORES          # 32 batches per core
NT = N // 128             # 16 n-tiles per batch
F32 = mybir.dt.float32
BF16 = mybir.dt.bfloat16
BF = ml_dtypes.bfloat16

# ---------------------------------------------------- tile tail-drain patch
# This walrus build rejects any instruction carrying >1 sync-wait command;
# TileContext's exit emits one SP drain waiting on every outstanding logical
# proc.  Split those waits across a chain of drains (semantically identical).
def _drain_and_barrier_split(self, tick_clock, wait_clock):
    drain_inst = self.nc.sync.drain()
    wait_clock.add_sem_waits(
        drain_inst.ins, ScopedClock({None: tick_clock.global_clock})
    )
    waits = list(drain_inst.ins.sync_info.on_wait)
    if len(waits) > 1:
        si = drain_inst.ins.sync_info
        si.on_wait = waits[:1]
        drain_inst.ins.sync_info = si
        for i in range(1, len(waits)):
            extra = self.nc.sync.drain()
            extra.ins.sync_info = bass_rust.SyncInfo(
                on_wait=waits[i : i + 1], on_update=[]
            )
    self.nc.all_engine_barrier()
    assert self.sems is not None
    popped = self.nc._tile_sem_poison_stack.pop()
    assert popped is self._sem_poison
    self.nc.clear_and_free_semaphores(list(self.sems.allocated().values()))
    self.nc.all_engine_barrier()


tile.TileContext._drain_and_barrier = _drain_and_barrier_split


def _bcast(ap, extra_count):
    """Append a step-0 (broadcast) innermost dim of `extra_count` to an AP."""
    return bass.AP(tensor=ap.tensor, offset=ap.offset, ap=[*ap.ap, [0, extra_count]])


# ------------------------------------------------------------- device program
def build_program():
    nc = bass.Bass()
    AOP = mybir.AluOpType
    ACT = mybir.ActivationFunctionType

    xT = nc.dram_tensor("xT", [KS, B], BF16, kind="ExternalInput")
    W3 = nc.dram_tensor("W3", [KS, 3 * D_OUT], BF16, kind="ExternalInput")
    biasb = nc.dram_tensor("biasb", [BS, 3 * D_OUT], F32, kind="ExternalInput")
    mem = nc.dram_tensor("mem", [BS, N, M], F32, kind="ExternalInput")
    ident = nc.dram_tensor("ident", [32, 32], BF16, kind="ExternalInput")
    newmem = nc.dram_tensor("newmem", [BS, N, M], F32, kind="ExternalOutput")
    wnew = nc.dram_tensor("wnew", [BS, H, N], F32, kind="ExternalOutput")

    KT = [128] * 12 + [16]  # k-tile sizes, sum = 1552

    with tile.TileContext(nc) as tc:
        consts = tc.tile_pool(name="consts", bufs=1)
        dram = tc.tile_pool(name="dram", bufs=1, space="DRAM")
        with consts, dram:
            # ---------------- phase A: partial y = x_shard @ W3_shard -------
            ident_sb = consts.tile([32, 32], BF16)
            nc.sync.dma_start(out=ident_sb, in_=ident[:, :])

            xT_sb, W3_sb = [], []
            off = 0
            for kk in KT:
                xt = consts.tile([128, B], BF16, tag=f"xt{off}")
                nc.sync.dma_start(out=xt[:kk, :], in_=xT[off : off + kk, :])
                wt = consts.tile([128, 3 * D_OUT], BF16, tag=f"wt{off}")
                nc.sync.dma_start(out=wt[:kk, :], in_=W3[off : off + kk, :])
                xT_sb.append(xt)
                W3_sb.append(wt)
                off += kk

            y_part = dram.tile([B, 3 * D_OUT], F32)
            with tc.tile_pool(name="ps_y", bufs=1, space="PSUM") as ps_y, \
                 tc.tile_pool(name="ysb", bufs=1) as ysb:
                for bh in range(2):
                    y_sb = ysb.tile([128, 3 * D_OUT], F32, tag=f"y{bh}")
                    for j in range(3):
                        yp = ps_y.tile([128, D_OUT], F32, tag=f"yp{bh}{j}")
                        for k, kk in enumerate(KT):
                            nc.tensor.matmul(
                                yp,
                                lhsT=xT_sb[k][:kk, 128 * bh : 128 * (bh + 1)],
                                rhs=W3_sb[k][:kk, D_OUT * j : D_OUT * (j + 1)],
                                start=(k == 0),
                                stop=(k == len(KT) - 1),
                            )
                        nc.vector.tensor_copy(
                            y_sb[:, D_OUT * j : D_OUT * (j + 1)], yp
                        )
                    nc.sync.dma_start(
                        out=y_part[128 * bh : 128 * (bh + 1), :], in_=y_sb
                    )

            # ---------------- reduce-scatter over the 8 cores ---------------
            y_rs = dram.tile([BS, 3 * D_OUT], F32)
            nc.gpsimd.collective_compute(
                "ReduceScatter",
                AOP.add,
                replica_groups=[list(range(NCORES))],
                ins=[y_part[:].opt()],
                outs=[y_rs[:].opt()],
            )

            # ---------------- phase B: k / e / a (tiny, 32 rows) ------------
            y32 = consts.tile([BS, 3 * D_OUT], F32)
            nc.sync.dma_start(out=y32, in_=y_rs[:])
            bias_sb = consts.tile([BS, 3 * D_OUT], F32)
            nc.sync.dma_start(out=bias_sb, in_=biasb[:, :])
            nc.vector.tensor_tensor(y32, y32, bias_sb, op=AOP.add)

            # k row norms over m-groups of 64
            nrm2k = consts.tile([BS, H], F32)
            ksq = consts.tile([BS, M], F32)
            for h in range(H):
                nc.vector.tensor_tensor_reduce(
                    out=ksq,
                    in0=y32[:, M * h : M * (h + 1)],
                    in1=y32[:, M * h : M * (h + 1)],
                    scale=1.0,
                    scalar=0.0,
                    op0=AOP.mult,
                    op1=AOP.add,
                    accum_out=nrm2k[:, h : h + 1],
                )
            knrm = consts.tile([BS, H], F32)
            nc.scalar.activation(knrm, nrm2k, ACT.Sqrt)
            rk = consts.tile([BS, H], F32)
            nc.vector.reciprocal(rk, knrm)

            k_norm = consts.tile([BS, H, M], BF16)
            nc.vector.tensor_tensor(
                k_norm,
                y32[:, : H * M].rearrange("p (h m) -> p h m", h=H),
                _bcast(rk[:, :], M),
                op=AOP.mult,
            )

            # e|a = sigmoid(y[:, 384:1152])
            ea_sig = consts.tile([BS, 2 * H * M], BF16)
            nc.scalar.activation(ea_sig, y32[:, H * M :], ACT.Sigmoid)

            # kT_all[64t+m, 32h+b] = k_norm[b, h, m], duplicated at t=0,1
            kT_sb = consts.tile([128, H * BS], BF16)
            with tc.tile_pool(name="ps_kt", bufs=1, space="PSUM") as ps_kt:
                kt_ps = ps_kt.tile([128, H * 32], BF16)
                for t in range(2):
                    for h in range(H):
                        nc.tensor.transpose(
                            out=kt_ps[64 * t : 64 * (t + 1), 32 * h : 32 * (h + 1)],
                            in_=k_norm[:, h, :],
                            identity=ident_sb[:, :],
                        )
                nc.vector.tensor_copy(kT_sb, kt_ps)

            # eaT_all[32q+h, 128w + 64*half + m] = ea_sig[b=4w+q, half, h, m]
            eaT = consts.tile([128, BS // 4 * 128], BF16)
            for b in range(BS):
                w, q = divmod(b, 4)
                src = ea_sig[b : b + 1, :].rearrange(
                    "p (x h m) -> p h x m", x=2, h=H
                )
                dst = eaT[
                    32 * q : 32 * q + H, 128 * w : 128 * (w + 1)
                ].rearrange("p (x m) -> p x m", x=2)
                nc.sync.dma_start(out=dst, in_=src)

            # ---------------- phase C: per-batch memory pipeline ------------
            with tc.tile_pool(name="mems", bufs=2) as memp, \
                 tc.tile_pool(name="small", bufs=2) as small, \
                 tc.tile_pool(name="mnorm", bufs=2) as mnp, \
                 tc.tile_pool(name="memt", bufs=2) as mtp, \
                 tc.tile_pool(name="soft", bufs=2) as soft, \
                 tc.tile_pool(name="newm", bufs=3) as newp, \
                 tc.tile_pool(name="ps_sc", bufs=1, space="PSUM") as ps_sc, \
                 tc.tile_pool(name="ps_ea", bufs=2, space="PSUM") as ps_ea:
                for wv in range(BS // 4):
                    sc_ps = ps_sc.tile([128, 2048], F32, tag="sc")
                    batch_data = []
                    for q in range(4):
                        b = 4 * wv + q
                        mem_sb = memp.tile([128, NT, M], F32, tag="mem")
                        nc.sync.dma_start(
                            out=mem_sb,
                            in_=mem[b, :, :].rearrange("(t p) m -> p t m", p=128),
                        )
                        sq_bf = memp.tile([128, NT, M], BF16, tag="sq")
                        nrm2 = small.tile([128, NT], F32, tag="nrm2")
                        for t in range(NT):
                            nc.vector.tensor_tensor_reduce(
                                out=sq_bf[:, t, :],
                                in0=mem_sb[:, t, :],
                                in1=mem_sb[:, t, :],
                                scale=1.0,
                                scalar=0.0,
                                op0=AOP.mult,
                                op1=AOP.add,
                                accum_out=nrm2[:, t : t + 1],
                            )
                        srt = small.tile([128, NT], F32, tag="srt")
                        nc.scalar.activation(srt, nrm2, ACT.Sqrt)
                        mrt = small.tile([128, NT], F32, tag="mrt")
                        nc.vector.reciprocal(mrt, srt)
                        m_norm = mnp.tile([128, NT, M], BF16, tag="mn")
                        nc.vector.tensor_tensor(
                            m_norm, mem_sb, _bcast(mrt[:, :], M), op=AOP.mult
                        )
                        memT = mtp.tile([128, 8, 128], BF16, tag="mt")
                        for u in range(8):
                            nc.sync.dma_start_transpose(
                                out=memT[:, u, :],
                                in_=m_norm[:, 2 * u : 2 * u + 2, :],
                            )
                        # scores: psum[32q+h, 1024t+512c+128u+n'] =
                        #   sum_m k_norm[b,h,m] * m_norm[b, 256(4c+u)+128t+n', m]
                        kcols = kT_sb[:, :].rearrange(
                            "p (h b2) -> p b2 h", b2=BS
                        )
                        for t in range(2):
                            for c in range(2):
                                nc.tensor.matmul(
                                    sc_ps[
                                        32 * q : 32 * q + H,
                                        1024 * t + 512 * c : 1024 * t + 512 * (c + 1),
                                    ],
                                    lhsT=kcols[64 * t : 64 * (t + 1), b, :],
                                    rhs=memT[
                                        64 * t : 64 * (t + 1), 4 * c : 4 * (c + 1), :
                                    ],
                                    start=True,
                                    stop=True,
                                )
                        batch_data.append((b, mem_sb, sq_bf))

                    # softmax over the wave's 4 batches (rows 32q+h)
                    esc = soft.tile([128, 2048], F32, tag="esc")
                    nc.scalar.activation(esc, sc_ps, ACT.Exp)
                    dsum = small.tile([128, 1], F32, tag="dsum")
                    nc.vector.reduce_sum(dsum, esc, axis=mybir.AxisListType.X)
                    rec = small.tile([128, 1], F32, tag="rec")
                    nc.vector.reciprocal(rec, dsum)
                    wn_f = soft.tile([128, 2048], F32, tag="wnf")
                    nc.vector.tensor_scalar_mul(wn_f, esc, rec[:, :])
                    wn_bf = soft.tile([128, 2048], BF16, tag="wnb")
                    nc.vector.tensor_scalar_mul(wn_bf, esc, rec[:, :])

                    for q in range(4):
                        b, mem_sb, sq_bf = batch_data[q]
                        # w_new out: n = 1024c + 256u + 128t + n'
                        nc.sync.dma_start(
                            out=wnew[b, :, :].rearrange(
                                "h (c u t n2) -> h t c u n2", c=2, u=4, t=2
                            ),
                            in_=wn_f[32 * q : 32 * q + H, :].rearrange(
                                "h (t c u n2) -> h t c u n2", t=2, c=2, u=4
                            ),
                        )
                        mem_r = mem_sb[:, :, :].rearrange(
                            "p (i t2) m -> p t2 i m", t2=2
                        )
                        sq_r = sq_bf[:, :, :].rearrange(
                            "p (i t2) m -> p t2 i m", t2=2
                        )
                        for e in range(2):
                            ea_ps = ps_ea.tile([128, 8, 128], F32, tag="ea")
                            for i in range(8):
                                sb = 8 * e + i
                                nc.tensor.matmul(
                                    ea_ps[:, i, :],
                                    lhsT=wn_bf[
                                        32 * q : 32 * q + H,
                                        128 * sb : 128 * (sb + 1),
                                    ],
                                    rhs=eaT[
                                        32 * q : 32 * q + H,
                                        128 * wv : 128 * (wv + 1),
                                    ],
                                    start=True,
                                    stop=True,
                                )
                            # update: new = mem - mem^2*E + A  (tau = 2i + e)
                            ybf = newp.tile([128, 8, M], BF16, tag="ybf")
                            nc.vector.scalar_tensor_tensor(
                                out=ybf,
                                in0=ea_ps[:, :, :M],
                                scalar=-1.0,
                                in1=sq_r[:, e, :, :],
                                op0=AOP.mult,
                                op1=AOP.mult,
                            )
                            cbf = newp.tile([128, 8, M], BF16, tag="cbf")
                            nc.vector.tensor_tensor(
                                cbf, ybf, ea_ps[:, :, M:], op=AOP.add
                            )
                            new_sb = newp.tile([128, 8, M], F32, tag="new")
                            nc.vector.tensor_tensor(
                                new_sb, mem_r[:, e, :, :], cbf, op=AOP.add
                            )
                            nc.sync.dma_start(
                                out=newmem[b, :, :].rearrange(
                                    "(i t2 p) m -> p t2 i m", t2=2, p=128
                                )[:, e, :, :],
                                in_=new_sb,
                            )
    return nc


_nc_cache = None


def _get_nc():
    global _nc_cache
    if _nc_cache is None:
        _nc_cache = build_program()
    return _nc_cache


# ------------------------------------------------------------------ host side
def kernel(inputs, memory, w, Wk, bk, We, be, Wa, ba):
    inputs = np.asarray(inputs, dtype=np.float32)
    memory = np.ascontiguousarray(np.asarray(memory, dtype=np.float32))
    w = np.asarray(w, dtype=np.float32)
    Wk, We, Wa = (np.asarray(a, dtype=np.float32) for a in (Wk, We, Wa))
    bk, be, ba = (np.asarray(a, dtype=np.float32) for a in (bk, be, ba))

    x = np.concatenate([w.reshape(B, -1), inputs], axis=1)          # [B, D_CAT]
    xT_bf = np.ascontiguousarray(x.T).astype(BF)                    # [D_CAT, B]
    W3 = np.concatenate([Wk, We, Wa], axis=1).astype(BF)            # [D_CAT, 1152]
    bias3 = np.concatenate([bk, be, ba]).astype(np.float32)
    biasb = np.ascontiguousarray(np.broadcast_to(bias3, (BS, 3 * D_OUT)))
    ident = np.eye(32, dtype=BF)

    in_maps = []
    for c in range(NCORES):
        in_maps.append(
            {
                "xT": np.ascontiguousarray(xT_bf[KS * c : KS * (c + 1)]),
                "W3": np.ascontiguousarray(W3[KS * c : KS * (c + 1)]),
                "biasb": biasb,
                "mem": np.ascontiguousarray(memory[BS * c : BS * (c + 1)]),
                "ident": ident,
            }
        )

    nc = _get_nc()
    res = run_bass_kernel_spmd(nc, in_maps, core_ids=list(range(NCORES)))
    new_memory = np.concatenate([res.results[c]["newmem"] for c in range(NCORES)], 0)
    w_new = np.concatenate([res.results[c]["wnew"] for c in range(NCORES)], 0)
    return new_memory, w_new


# revision 19
# speedup vs baseline: 1.2262x; 1.2262x over previous
"""NTM-style scatter-memory kernel for Trainium2, 8 NeuronCores.

Strategy (K-sharded matmul + batch-sharded memory update):
  - The three Dense matmuls x @ [Wk|We|Wa] (K = 12416) are sharded over the
    contraction dim: core c holds rows [1552c, 1552(c+1)) of the weights and
    the matching columns of x^T, computes a partial [256, 1152] product, and a
    ReduceScatter(+) hands core c the full-sum rows for its 32 batches.
    This avoids replicating the ~19 MB weight matrices on every core.
  - Everything else (cosine addressing, softmax, erase/add, memory update) is
    batch-local: core c owns batches [32c, 32c+32).
Host-side: shard/concat/cast prep in numpy; device kernel does all FLOPs.
"""

import numpy as np
import ml_dtypes

import bass_rust
import concourse.bass as bass
import concourse.tile as tile
from concourse import mybir
from concourse.bass_utils import run_bass_kernel_spmd
from concourse.vector_clock import ScopedClock

# ---------------------------------------------------------------- dimensions
B, H, N, M, D_IN = 256, 6, 2048, 64, 128
D_CAT = H * N + D_IN      # 12416
D_OUT = H * M             # 384
NCORES = 8
KS = D_CAT // NCORES      # 1552 contraction rows per core
BS = B // NCORES          # 32 batches per core
NT = N // 128             # 16 n-tiles per batch
F32 = mybir.dt.float32
BF16 = mybir.dt.bfloat16
BF = ml_dtypes.bfloat16

# ------------------------------------------------- sync-wait splitting pass
# This walrus build rejects any instruction carrying >1 sync-wait command.
# After Tile scheduling, hoist all but one wait of each instruction onto
# no-op predecessors on the same engine (engines execute in program order,
# so waiting on the nops first is semantically identical).
_split_ctr = [0]


def split_excess_waits(nc):
    for fn in nc.m.functions:
        for bb in fn.blocks:
            insts = bb.instructions
            out, changed = [], False
            for ins in insts:
                si = ins.sync_info
                if si is not None and len(si.on_wait) > 1:
                    waits = list(si.on_wait)
                    eng = ins.engine
                    for wsub in waits[:-1]:
                        _split_ctr[0] += 1
                        nop = mybir.InstNoOp(
                            name=f"I-wsplit-{_split_ctr[0]}",
                            engine=eng,
                            bass_nofuse=True,
                            sync_info=mybir.SyncInfo(
                                on_wait=[wsub], on_update=[]
                            ),
                        )
                        out.append(nop)
                    si.on_wait = [waits[-1]]
                    ins.sync_info = si
                    changed = True
                out.append(ins)
            if changed:
                bb.instructions = out


def _bcast(ap, extra_count):
    """Append a step-0 (broadcast) innermost dim of `extra_count` to an AP."""
    return bass.AP(tensor=ap.tensor, offset=ap.offset, ap=[*ap.ap, [0, extra_count]])


# ------------------------------------------------------------- device program
def build_program():
    nc = bass.Bass(num_devices=NCORES)
    AOP = mybir.AluOpType
    ACT = mybir.ActivationFunctionType

    xT = nc.dram_tensor("xT", [KS, B], BF16, kind="ExternalInput")
    W3 = nc.dram_tensor("W3", [KS, 3 * D_OUT], BF16, kind="ExternalInput")
    biasb = nc.dram_tensor("biasb", [BS, 3 * D_OUT], F32, kind="ExternalInput")
    mem = nc.dram_tensor("mem", [BS, N, M], F32, kind="ExternalInput")
    ident = nc.dram_tensor("ident", [32, 32], BF16, kind="ExternalInput")
    newmem = nc.dram_tensor("newmem", [BS, N, M], F32, kind="ExternalOutput")
    wnew = nc.dram_tensor("wnew", [BS, H, N], F32, kind="ExternalOutput")

    KT = [128] * 12 + [16]  # k-tile sizes, sum = 1552

    with tile.TileContext(nc) as tc:
        with tc.tile_pool(name="consts", bufs=1) as consts, \
             tc.tile_pool(name="dram", bufs=1, space="DRAM") as dram:
            # ---------------- phase A: partial y = x_shard @ W3_shard -------
            ident_sb = consts.tile([32, 32], BF16)
            nc.sync.dma_start(out=ident_sb, in_=ident[:, :])

            xT_sb, W3_sb = [], []
            off = 0
            for kk in KT:
                xt = consts.tile([128, B], BF16, tag=f"xt{off}")
                nc.sync.dma_start(out=xt[:kk, :], in_=xT[off : off + kk, :])
                wt = consts.tile([128, 3 * D_OUT], BF16, tag=f"wt{off}")
                nc.sync.dma_start(out=wt[:kk, :], in_=W3[off : off + kk, :])
                xT_sb.append(xt)
                W3_sb.append(wt)
                off += kk

            y_part = dram.tile([B, 3 * D_OUT], F32)
            with tc.tile_pool(name="ps_y", bufs=1, space="PSUM") as ps_y, \
                 tc.tile_pool(name="ysb", bufs=1) as ysb:
                for bh in range(2):
                    y_sb = ysb.tile([128, 3 * D_OUT], F32, tag=f"y{bh}")
                    for j in range(3):
                        yp = ps_y.tile([128, D_OUT], F32, tag=f"yp{bh}{j}")
                        for k, kk in enumerate(KT):
                            nc.tensor.matmul(
                                yp,
                                lhsT=xT_sb[k][:kk, 128 * bh : 128 * (bh + 1)],
                                rhs=W3_sb[k][:kk, D_OUT * j : D_OUT * (j + 1)],
                                start=(k == 0),
                                stop=(k == len(KT) - 1),
                            )
                        nc.vector.tensor_copy(
                            y_sb[:, D_OUT * j : D_OUT * (j + 1)], yp
                        )
                    nc.sync.dma_start(
                        out=y_part[128 * bh : 128 * (bh + 1), :], in_=y_sb
                    )

            # ---------------- reduce-scatter over the 8 cores ---------------
            y_rs = dram.tile([BS, 3 * D_OUT], F32)
            nc.gpsimd.collective_compute(
                "ReduceScatter",
                AOP.add,
                replica_groups=[list(range(NCORES))],
                ins=[y_part[:].opt()],
                outs=[y_rs[:].opt()],
            )

            # ---------------- phase B: k / e / a (tiny, 32 rows) ------------
            y32 = consts.tile([BS, 3 * D_OUT], F32)
            nc.sync.dma_start(out=y32, in_=y_rs[:])
            bias_sb = consts.tile([BS, 3 * D_OUT], F32)
            nc.sync.dma_start(out=bias_sb, in_=biasb[:, :])
            nc.vector.tensor_tensor(y32, y32, bias_sb, op=AOP.add)

            # k row norms over m-groups of 64
            ksq = consts.tile([BS, H * M], F32)
            nc.vector.tensor_tensor(ksq, y32[:, : H * M], y32[:, : H * M], op=AOP.mult)
            nrm2k = consts.tile([BS, H], F32)
            nc.vector.tensor_reduce(
                nrm2k,
                ksq[:, :].rearrange("p (h m) -> p h m", h=H),
                axis=mybir.AxisListType.X,
                op=AOP.add,
            )
            knrm = consts.tile([BS, H], F32)
            nc.scalar.activation(knrm, nrm2k, ACT.Sqrt)
            rk = consts.tile([BS, H], F32)
            nc.vector.reciprocal(rk, knrm)

            k_norm = consts.tile([BS, H, M], BF16)
            nc.vector.tensor_tensor(
                k_norm,
                y32[:, : H * M].rearrange("p (h m) -> p h m", h=H),
                _bcast(rk[:, :], M),
                op=AOP.mult,
            )

            # e|a = sigmoid(y[:, 384:1152])
            ea_sig = consts.tile([BS, 2 * H * M], BF16)
            nc.scalar.activation(ea_sig, y32[:, H * M :], ACT.Sigmoid)

            # kT_all[64t+m, 32h+b] = k_norm[b, h, m], duplicated at t=0,1
            kT_sb = consts.tile([128, H * BS], BF16)
            with tc.tile_pool(name="ps_kt", bufs=1, space="PSUM") as ps_kt:
                kt_ps = ps_kt.tile([128, H * 32], BF16)
                for t in range(2):
                    for h in range(H):
                        nc.tensor.transpose(
                            out=kt_ps[64 * t : 64 * (t + 1), 32 * h : 32 * (h + 1)],
                            in_=k_norm[:, h, :],
                            identity=ident_sb[:, :],
                            tile_position=(0, 64 * t),
                        )
                nc.vector.tensor_copy(kT_sb, kt_ps)

            # eaT_all[32q+h, 128w + 64*half + m] = ea_sig[b=4w+q, half, h, m]
            eaT = consts.tile([128, BS // 4 * 128], BF16)
            for b in range(BS):
                w, q = divmod(b, 4)
                for half in range(2):
                    src = ea_sig[
                        b : b + 1, 384 * half : 384 * (half + 1)
                    ].rearrange("p (h m) -> p h m", h=H)
                    dst = eaT[
                        32 * q : 32 * q + H,
                        128 * w + 64 * half : 128 * w + 64 * (half + 1),
                    ]
                    nc.sync.dma_start(out=dst, in_=src)

            # ---------------- phase C: per-batch memory pipeline ------------
            with tc.tile_pool(name="mems", bufs=6) as memp, \
                 tc.tile_pool(name="small", bufs=2) as small, \
                 tc.tile_pool(name="mnorm", bufs=2) as mnp, \
                 tc.tile_pool(name="memt", bufs=2) as mtp, \
                 tc.tile_pool(name="soft", bufs=2) as soft, \
                 tc.tile_pool(name="newm", bufs=3) as newp, \
                 tc.tile_pool(name="ps_sc", bufs=1, space="PSUM") as ps_sc, \
                 tc.tile_pool(name="ps_ea", bufs=2, space="PSUM") as ps_ea:
                for wv in range(BS // 4):
                    sc_ps = ps_sc.tile([128, 2048], F32, tag="sc")
                    batch_data = []
                    for q in range(4):
                        b = 4 * wv + q
                        mem_sb = memp.tile([128, NT, M], F32, tag="mem")
                        nc.sync.dma_start(
                            out=mem_sb,
                            in_=mem[b, :, :].rearrange("(t p) m -> p t m", p=128),
                        )
                        sq_bf = memp.tile([128, NT, M], BF16, tag="sq")
                        nc.vector.tensor_tensor(
                            sq_bf, mem_sb, mem_sb, op=AOP.mult
                        )
                        nrm2 = small.tile([128, NT], F32, tag="nrm2")
                        nc.vector.tensor_reduce(
                            nrm2,
                            sq_bf,
                            axis=mybir.AxisListType.X,
                            op=AOP.add,
                        )
                        srt = small.tile([128, NT], F32, tag="srt")
                        nc.scalar.activation(srt, nrm2, ACT.Sqrt)
                        mrt = small.tile([128, NT], F32, tag="mrt")
                        nc.vector.reciprocal(mrt, srt)
                        m_norm = mnp.tile([128, NT, M], BF16, tag="mn")
                        nc.vector.tensor_tensor(
                            m_norm, mem_sb, _bcast(mrt[:, :], M), op=AOP.mult
                        )
                        memT = mtp.tile([128, 8, 128], BF16, tag="mt")
                        for u in range(8):
                            nc.sync.dma_start_transpose(
                                out=memT[:, u, :],
                                in_=m_norm[:, 2 * u : 2 * u + 2, :],
                            )
                        # scores: psum[32q+h, 1024t+512c+128u+n'] =
                        #   sum_m k_norm[b,h,m] * m_norm[b, 256(4c+u)+128t+n', m]
                        kcols = kT_sb[:, :].rearrange(
                            "p (h b2) -> p b2 h", b2=BS
                        )
                        for t in range(2):
                            for c in range(2):
                                nc.tensor.matmul(
                                    sc_ps[
                                        32 * q : 32 * q + H,
                                        1024 * t + 512 * c : 1024 * t + 512 * (c + 1),
                                    ],
                                    lhsT=kcols[64 * t : 64 * (t + 1), b, :],
                                    rhs=memT[
                                        64 * t : 64 * (t + 1), 4 * c : 4 * (c + 1), :
                                    ],
                                    start=True,
                                    stop=True,
                                    tile_position=(64 * t, 32 * q),
                                )
                        batch_data.append((b, mem_sb, sq_bf))

                    # softmax over the wave's 4 batches (rows 32q+h)
                    esc = soft.tile([128, 2048], F32, tag="esc")
                    nc.scalar.activation(esc, sc_ps, ACT.Exp)
                    dsum = small.tile([128, 1], F32, tag="dsum")
                    nc.vector.reduce_sum(dsum, esc, axis=mybir.AxisListType.X)
                    rec = small.tile([128, 1], F32, tag="rec")
                    nc.vector.reciprocal(rec, dsum)
                    wn_f = soft.tile([128, 2048], F32, tag="wnf")
                    nc.vector.tensor_scalar_mul(wn_f, esc, rec[:, :])
                    wn_bf = soft.tile([128, 2048], BF16, tag="wnb")
                    nc.vector.tensor_scalar_mul(wn_bf, esc, rec[:, :])

                    for q in range(4):
                        b, mem_sb, sq_bf = batch_data[q]
                        # w_new out: n = 256*(4c+u) + 128t + n'
                        for t in range(2):
                            nc.sync.dma_start(
                                out=wnew[b, :, :].rearrange(
                                    "h (k t2 n2) -> h t2 k n2", t2=2, n2=128
                                )[:, t],
                                in_=wn_f[
                                    32 * q : 32 * q + H,
                                    1024 * t : 1024 * (t + 1),
                                ].rearrange("h (k n2) -> h k n2", n2=128),
                            )
                        mem_r = mem_sb[:, :, :].rearrange(
                            "p (i t2) m -> p t2 i m", t2=2
                        )
                        sq_r = sq_bf[:, :, :].rearrange(
                            "p (i t2) m -> p t2 i m", t2=2
                        )
                        for e in range(2):
                            ea_ps = ps_ea.tile([128, 8, 128], F32, tag="ea")
                            for i in range(8):
                                sb = 8 * e + i
                                nc.tensor.matmul(
                                    ea_ps[:, i, :],
                                    lhsT=wn_bf[
                                        32 * q : 32 * q + H,
                                        128 * sb : 128 * (sb + 1),
                                    ],
                                    rhs=eaT[
                                        32 * q : 32 * q + H,
                                        128 * wv : 128 * (wv + 1),
                                    ],
                                    start=True,
                                    stop=True,
                                    tile_position=(32 * q, 0),
                                )
                            # update: new = mem - mem^2*E + A  (tau = 2i + e)
                            ybf = newp.tile([128, 8, M], BF16, tag="ybf")
                            nc.vector.scalar_tensor_tensor(
                                out=ybf,
                                in0=ea_ps[:, :, :M],
                                scalar=-1.0,
                                in1=sq_r[:, e, :, :],
                                op0=AOP.mult,
                                op1=AOP.mult,
                            )
                            cbf = newp.tile([128, 8, M], BF16, tag="cbf")
                            nc.vector.tensor_tensor(
                                cbf, ybf, ea_ps[:, :, M:], op=AOP.add
                            )
                            new_sb = newp.tile([128, 8, M], F32, tag="new")
                            nc.vector.tensor_tensor(
                                new_sb, mem_r[:, e, :, :], cbf, op=AOP.add
                            )
                            nc.sync.dma_start(
                                out=newmem[b, :, :].rearrange(
                                    "(i t2 p) m -> p t2 i m", t2=2, p=128
                                )[:, e, :, :],
                                in_=new_sb,
                            )
    split_excess_waits(nc)
    return nc


_nc_cache = None


def _get_nc():
    global _nc_cache
    if _nc_cache is None:
        _nc_cache = build_program()
    return _nc_cache


# ------------------------------------------------------------------ host side
def prepare_in_maps(inputs, memory, w, Wk, bk, We, be, Wa, ba):
    inputs = np.asarray(inputs, dtype=np.float32)
    memory = np.ascontiguousarray(np.asarray(memory, dtype=np.float32))
    w = np.asarray(w, dtype=np.float32)
    Wk, We, Wa = (np.asarray(a, dtype=np.float32) for a in (Wk, We, Wa))
    bk, be, ba = (np.asarray(a, dtype=np.float32) for a in (bk, be, ba))

    x = np.concatenate([w.reshape(B, -1), inputs], axis=1)          # [B, D_CAT]
    xT_bf = np.ascontiguousarray(x.T).astype(BF)                    # [D_CAT, B]
    W3 = np.concatenate([Wk, We, Wa], axis=1).astype(BF)            # [D_CAT, 1152]
    bias3 = np.concatenate([bk, be, ba]).astype(np.float32)
    biasb = np.ascontiguousarray(np.broadcast_to(bias3, (BS, 3 * D_OUT)))
    ident = np.eye(32, dtype=BF)

    in_maps = []
    for c in range(NCORES):
        in_maps.append(
            {
                "xT": np.ascontiguousarray(xT_bf[KS * c : KS * (c + 1)]),
                "W3": np.ascontiguousarray(W3[KS * c : KS * (c + 1)]),
                "biasb": biasb,
                "mem": np.ascontiguousarray(memory[BS * c : BS * (c + 1)]),
                "ident": ident,
            }
        )
    return in_maps


def kernel(**inputs):
    in_maps = prepare_in_maps(**inputs)
    nc = _get_nc()
    res = run_bass_kernel_spmd(nc, in_maps, core_ids=list(range(NCORES)))
    kernel.last_results = res
    new_memory = np.concatenate([res.results[c]["newmem"] for c in range(NCORES)], 0)
    w_new = np.concatenate([res.results[c]["wnew"] for c in range(NCORES)], 0)
    return new_memory, w_new
